# revision 21
# baseline (speedup 1.0000x reference)
"""Trainium2 Bass kernel for the ICP depth-term loss (bidirectional masked
nearest-neighbour correspondence + mean distance).

Semantics (validated vs reference to ~7e-8 in fp64): for each query q against
refs {r}, dv = min d2 over cos-valid refs, dmin = min d2 over all refs,
picked = dv if dv < TH2 else dmin, loss = mean(sqrt(picked)) summed over both
directions.

Key structural idea: d2 and the normal-cosine grid are SYMMETRIC, so one pass
over the [depth x verts] grid serves BOTH directions.  The depth cloud is
sharded across the 8 cores (6272 points each); each grid tile is
[128 depth (partitions), 1024 verts (free)]:

  PE   psD = d2 (K=15 hi/lo-split features), psC = B*(COS-cos) (K=11)
  ACT  d2b = bf16 copy of psD  -> right half of a [128,2048] "cat" tile
  DVE  tensor_tensor_reduce: m = max(d2b, psC) -> left half of cat,
       free-axis min accum -> direction-B masked min (per depth point)
  DVE  TT min(runA[vst], cat): ONE 2x-mode op accumulates direction-A's
       masked AND plain running minima over depth tiles
  DVE  TT min(runBM, d2b): direction-B plain running min over vert tiles

The masked min uses max(d2, B*(COS-cos)) which equals d2 for cos-valid pairs
and is huge otherwise - no relu / extra activation pass needed.  There is no
1x-mode tensor_reduce in the hot loop (only per-dt [128,7]/[128,1024] finals).
Direction A's partition-axis collapse is done once at the end via PE
transposes + small psum reduces; the cross-core merge, threshold select, sqrt
and mean (O(N+M) epilogue) run on host.
"""

import math

import numpy as np

import concourse.bass as bass  # noqa: F401  (engine types referenced via nc)
import concourse.bacc as bacc
import concourse.tile as tile
from concourse import mybir
from concourse.bass_utils import run_bass_kernel_spmd
from concourse.tile_rust import add_dep_helper

N_VERTS = 6890
M_DEPTH = 50000
N_CORES = 8

import os as _os

DQ = 6272            # depth points per core (49 tiles x 128 partitions)
NDT = 49
RV = 7168            # verts padded
FD = int(_os.environ.get("BASS_V6_FD", "1024"))  # free dim per super-tile
NVST = RV // FD      # vert super-tiles
NB = FD // 128       # 128-blocks per super-tile (A-epilogue transposes)

TH2 = 0.05 * 0.05
COS = math.cos(math.pi / 12.0)
B = 1.0e4            # cos-penalty scale: pen = B*(COS - cos)
BIG = 1.0e6          # min-reduce init; > any masked value (max pen = 2B)
PAD = 60.0           # padding coordinate: d2 >= 10800 vs any real point

F32 = mybir.dt.float32
BF16 = mybir.dt.bfloat16
AX = mybir.AxisListType.X
OP = mybir.AluOpType


def _build(repeat=1):
    nc = bacc.Bacc("TRN2")

    depD = nc.declare_dram_parameter("depD", [15, DQ], BF16, isOutput=False)
    depC = nc.declare_dram_parameter("depC", [11, DQ], BF16, isOutput=False)
    vertD = nc.declare_dram_parameter("vertD", [15, RV], BF16, isOutput=False)
    vertC = nc.declare_dram_parameter("vertC", [11, RV], BF16, isOutput=False)
    ident = nc.declare_dram_parameter("ident", [128, 128], F32, isOutput=False)
    outA_d = nc.declare_dram_parameter("outA", [128, 112], F32, isOutput=True)
    outB_d = nc.declare_dram_parameter("outB", [128, 99], F32, isOutput=True)

    from contextlib import ExitStack
    with ExitStack() as ctx:
        tc = ctx.enter_context(tile.TileContext(nc))
        singles = ctx.enter_context(tc.tile_pool(name="singles", bufs=1))
        work = ctx.enter_context(tc.tile_pool(name="work", bufs=3))
        psD_pool = ctx.enter_context(tc.tile_pool(name="psD", bufs=2, space="PSUM"))
        psC_pool = ctx.enter_context(tc.tile_pool(name="psC", bufs=1, space="PSUM"))
        psT_pool = ctx.enter_context(tc.tile_pool(name="psT", bufs=2, space="PSUM"))

        # Resident inputs
        depD_sb = singles.tile([15, DQ], BF16)
        depC_sb = singles.tile([11, DQ], BF16)
        vertD_sb = singles.tile([15, RV], BF16)
        vertC_sb = singles.tile([11, RV], BF16)
        ident32_sb = singles.tile([128, 128], F32)
        nc.gpsimd.dma_start(out=depD_sb, in_=depD[:, :])
        nc.gpsimd.dma_start(out=depC_sb, in_=depC[:, :])
        nc.gpsimd.dma_start(out=vertD_sb, in_=vertD[:, :])
        nc.gpsimd.dma_start(out=vertC_sb, in_=vertC[:, :])
        nc.gpsimd.dma_start(out=ident32_sb, in_=ident[:, :])

        outA_sb = singles.tile([128, 112], F32)
        outB_sb = singles.tile([128, 99], F32)
        runB = singles.tile([128, 2 * FD], BF16)
        runA = [singles.tile([128, 2 * FD], BF16, tag=f"runA{i}",
                             name=f"runA{i}")
                for i in range(NVST)]

        # One tiny PE matmul per resident DMA so the PE observes each DMA's
        # semaphore once; later matmuls ride on PE program order (a Matmult
        # carries at most one semaphore wait).  The dummy target borrows a
        # psD-ring buffer (returned to the ring before the main loop).
        dumT = psD_pool.tile([128, FD], F32, tag="psD")
        observers = []
        for sb in (depD_sb, depC_sb, vertD_sb, vertC_sb, ident32_sb):
            observers.append(
                nc.tensor.matmul(dumT[0:1, 0:1], sb[:, 0:1], sb[:, 0:1],
                                 start=True, stop=True, skip_group_check=True))
        nc.vector.memset(outB_sb[:, 98:99], 0.0)
        nc.vector.tensor_copy(outB_sb[0:1, 98:99], dumT[0:1, 0:1])

        for _rep in range(repeat):
            for dt in range(NDT):
                ds = slice(dt * 128, (dt + 1) * 128)
                for vst in range(NVST):
                    psC = psC_pool.tile([128, FD], F32, tag="psC")
                    psD = psD_pool.tile([128, FD], F32, tag="psD")
                    mmC0 = mmD0 = None
                    for h in range(0, FD, 512):
                        hs = slice(h, min(h + 512, FD))
                        vs = slice(vst * FD + h, vst * FD + min(h + 512, FD))
                        mmC = nc.tensor.matmul(psC[:, hs], depC_sb[:, ds],
                                               vertC_sb[:, vs],
                                               start=True, stop=True)
                        mmD = nc.tensor.matmul(psD[:, hs], depD_sb[:, ds],
                                               vertD_sb[:, vs],
                                               start=True, stop=True)
                        if mmC0 is None:
                            mmC0, mmD0 = mmC, mmD
                    if _rep == 0 and dt == 0 and vst == 0:
                        for obs in observers:
                            add_dep_helper(mmC0.ins, obs.ins, sync=False,
                                           reason="observe DMA before matmul")
                            add_dep_helper(mmD0.ins, obs.ins, sync=False,
                                           reason="observe DMA before matmul")

                    cat = work.tile([128, 2 * FD], BF16, tag="cat")
                    d2b = cat[:, FD:2 * FD]
                    mview = cat[:, 0:FD]
                    nc.scalar.activation(d2b, psD,
                                         mybir.ActivationFunctionType.Copy)
                    # masked values: m = max(d2, pen); pen<=0 iff cos-valid
                    nc.vector.tensor_tensor(mview, d2b, psC, op=OP.max)
                    if dt == 0:
                        nc.vector.tensor_copy(runA[vst], cat)
                    else:
                        nc.vector.tensor_tensor(runA[vst], runA[vst], cat,
                                                op=OP.min)
                    if vst == 0:
                        nc.vector.tensor_copy(runB, cat)
                    else:
                        nc.vector.tensor_tensor(runB, runB, cat, op=OP.min)
                # direction-B second level for this depth tile
                nc.vector.tensor_reduce(
                    out=outB_sb[:, dt:dt + 1],
                    in_=runB[:, 0:FD], axis=AX, op=OP.min)
                nc.vector.tensor_reduce(
                    out=outB_sb[:, NDT + dt:NDT + dt + 1],
                    in_=runB[:, FD:2 * FD], axis=AX, op=OP.min)

            # direction-A epilogue: collapse the depth partition axis.
            # PE transpose requires out.dtype == in.dtype and PSUM is fp32,
            # so stage runA through an fp32 SBUF copy first.
            import os as _os
            if _os.environ.get("BASS_V6_NO_EPILOGUE"):
                nc.vector.memset(outA_sb[:, :], 0.0)
            else:
                for vst in range(NVST):
                    runA32 = work.tile([128, 2 * FD], F32, tag="runA32",
                                       bufs=2)
                    nc.scalar.activation(runA32, runA[vst],
                                         mybir.ActivationFunctionType.Copy)
                    for off, cbase in ((0, 0), (FD, 56)):
                        for b in range(NB):
                            tr = psT_pool.tile([128, 128], F32, tag="psT")
                            nc.tensor.transpose(
                                tr,
                                runA32[:, off + b * 128: off + (b + 1) * 128],
                                ident32_sb)
                            c0 = cbase + vst * NB + b
                            nc.vector.tensor_reduce(
                                out=outA_sb[:, c0:c0 + 1], in_=tr,
                                axis=AX, op=OP.min)

        nc.gpsimd.dma_start(out=outA_d[:, :], in_=outA_sb)
        nc.gpsimd.dma_start(out=outB_d[:, :], in_=outB_sb)

    nc.finalize()
    return nc


def _pack_inputs(depth_vmap, depth_nmap, verts_src, normal_src):
    import ml_dtypes
    BF = ml_dtypes.bfloat16

    d = np.ascontiguousarray(np.asarray(depth_vmap, dtype=np.float32))
    nd = np.ascontiguousarray(np.asarray(depth_nmap, dtype=np.float32))
    v = np.ascontiguousarray(np.asarray(verts_src, dtype=np.float32))
    nv = np.ascontiguousarray(np.asarray(normal_src, dtype=np.float32))

    def split(x):
        hi = x.astype(BF).astype(np.float32)
        lo = (x - hi).astype(BF).astype(np.float32)
        return hi, lo

    dep = np.full((N_CORES * DQ, 3), PAD, np.float32); dep[:M_DEPTH] = d
    depn = np.zeros((N_CORES * DQ, 3), np.float32); depn[:, 0] = 1.0
    depn[:M_DEPTH] = nd
    vert = np.full((RV, 3), PAD, np.float32); vert[:N_VERTS] = v
    vertn = np.zeros((RV, 3), np.float32); vertn[:, 0] = 1.0
    vertn[:N_VERTS] = nv

    # d2 features: K=15 hi/lo split, d2 = |q|^2 + |r|^2 - 2 q.r
    q = dep.T; qh, ql = split(q)
    q2 = (dep.astype(np.float64) ** 2).sum(1).astype(np.float32)
    q2h, q2l = split(q2)
    depD = np.zeros((15, N_CORES * DQ), np.float32)
    depD[0:3] = qh; depD[3] = q2h; depD[4] = 1.0
    depD[5:8] = ql; depD[8] = q2l; depD[9] = 0.0
    depD[10:13] = qh; depD[13] = 0.0; depD[14] = 1.0

    t = -2.0 * vert.T; th, tl = split(t)
    r2 = (vert.astype(np.float64) ** 2).sum(1).astype(np.float32)
    r2h, r2l = split(r2)
    vertD = np.zeros((15, RV), np.float32)
    vertD[0:3] = th; vertD[3] = 1.0; vertD[4] = r2h
    vertD[5:8] = th; vertD[8] = 1.0; vertD[9] = r2h
    vertD[10:13] = tl; vertD[13] = 0.0; vertD[14] = r2l

    # cos-penalty features: pen = B*COS - B*(nd.nv), hi/lo split products
    ndh, ndl = split(depn.T)
    depC = np.zeros((11, N_CORES * DQ), np.float32)
    depC[0:3] = ndh; depC[3:6] = ndh; depC[6:9] = ndl
    depC[9] = 1.0; depC[10] = 1.0
    nvh, nvl = split(vertn.T)
    bias = np.float32(B * COS)
    bh = np.float32(BF(bias)); bl = np.float32(BF(np.float32(bias - bh)))
    vertC = np.zeros((11, RV), np.float32)
    vertC[0:3] = -B * nvh; vertC[3:6] = -B * nvl; vertC[6:9] = -B * nvh
    vertC[9] = bh; vertC[10] = bl

    ident = np.eye(128, dtype=np.float32)
    vertD_bf = vertD.astype(BF); vertC_bf = vertC.astype(BF)
    depD_bf = depD.astype(BF); depC_bf = depC.astype(BF)

    in_maps = []
    for c in range(N_CORES):
        cs = slice(c * DQ, (c + 1) * DQ)
        in_maps.append({
            "depD": np.ascontiguousarray(depD_bf[:, cs]),
            "depC": np.ascontiguousarray(depC_bf[:, cs]),
            "vertD": vertD_bf,
            "vertC": vertC_bf,
            "ident": ident,
        })
    return in_maps


_CACHE = {}


def _cache_nc():
    if "nc" not in _CACHE:
        _CACHE["nc"] = _build()
    return _CACHE["nc"]


def kernel(depth_vmap, depth_nmap, verts_src, normal_src, k, _cache=_CACHE):
    in_maps = _pack_inputs(depth_vmap, depth_nmap, verts_src, normal_src)
    res = run_bass_kernel_spmd(_cache_nc(), in_maps,
                               core_ids=list(range(N_CORES)))

    allV = np.empty((N_CORES, RV), np.float32)
    allM = np.empty((N_CORES, RV), np.float32)
    dvB = np.empty(N_CORES * DQ, np.float32)
    dmB = np.empty(N_CORES * DQ, np.float32)
    for c, r in enumerate(res.results):
        outA = r["outA"]; outB = r["outB"]
        allV[c] = outA[:, 0:56].reshape(128, NVST, NB).transpose(1, 2, 0).reshape(RV)
        allM[c] = outA[:, 56:112].reshape(128, NVST, NB).transpose(1, 2, 0).reshape(RV)
        dvB[c * DQ:(c + 1) * DQ] = outB[:, 0:NDT].T.reshape(DQ)
        dmB[c * DQ:(c + 1) * DQ] = outB[:, NDT:2 * NDT].T.reshape(DQ)

    dvA = allV.min(0)[:N_VERTS]; dmA = allM.min(0)[:N_VERTS]
    pickA = np.where(dvA < TH2, dvA, dmA)
    lossA = np.sqrt(np.maximum(pickA, 0, dtype=np.float64)).mean()
    dvB = dvB[:M_DEPTH]; dmB = dmB[:M_DEPTH]
    pickB = np.where(dvB < TH2, dvB, dmB)
    lossB = np.sqrt(np.maximum(pickB, 0, dtype=np.float64)).mean()
    return np.float32(lossA + lossB)


if __name__ == "__main__":
    rng = np.random.default_rng(0)
    d = rng.standard_normal((M_DEPTH, 3)).astype(np.float32)
    nd = rng.standard_normal((M_DEPTH, 3)).astype(np.float32)
    nd /= np.linalg.norm(nd, axis=1, keepdims=True)
    v = rng.standard_normal((N_VERTS, 3)).astype(np.float32)
    nv = rng.standard_normal((N_VERTS, 3)).astype(np.float32)
    nv /= np.linalg.norm(nv, axis=1, keepdims=True)
    print(kernel(d, nd, v, nv, 32))
